# revision 18
# baseline (speedup 1.0000x reference)
"""Causal self-attention (B=4, T=2048, D=1024, H=16, rope) on 8 trn2 cores.

Sharding: DP over batch (4) x TP over heads (2 groups of 8). Core c handles
batch c//2, heads [8*(c%2), 8*(c%2)+8). Host pre-slices/transposes weights,
sums the two partial out-projections per batch afterwards.

Per-core kernel (v8): bf16 matmul datapath (PSUM accumulation f32).
  - RoPE via a second projection against sign-folded swapped weights
    (wqksT[:,f] = sign(f) * wqkT[:,partner(f)]): q_rot = q*cos + q_swap*sin
    becomes three DVE ops per m-pair reading the projection PSUM directly --
    no ACT staging copy, no gpsimd partition-offset muls.
  - v projected token-major in its own pass (x re-streamed) [tok, 65*8] with
    a ones column per head so softmax denominators ride the attention matmul.
  - scores transposed (keys on partitions): S_T = k_blk.T @ q per 128-key
    block; 2-head-wide tiles, PSUM triple-buffered, S/exp/O software-
    pipelined (S of block idx+2 issues before O of block idx so the PE never
    idle-waits on the ACT exp); one ACT Exp per block (scale=1/8 folded) ->
    bf16. Causal: fully-masked blocks skipped, diagonal blocks column-trimmed
    to (0,128,256,384); the diagonal triangle is masked post-exp by a DVE mul
    into a SEPARATE tile so exps only ever wait on the PE semaphore.
  - O_T[d, q] accumulates over key blocks in PSUM; row 64 = softmax sums;
    divide via ones-matmul partition broadcast (f32r) + DVE reciprocal/mul.
  - out-projection per 512-token block -> yT [1024, 2048] partial (f32).
"""
import sys

for _p in ("/opt/trn_rl_repo", "/root/.axon_site/_ro/trn_rl_repo"):
    if _p not in sys.path:
        sys.path.insert(0, _p)

import numpy as np
import ml_dtypes
import concourse.bass as bass
import concourse.mybir as mybir
import concourse.tile as tile
from concourse.bass_utils import run_bass_kernel_spmd

D_MODEL = 1024
N_HEADS = 16
HEAD_DIM = 64
T = 2048
B = 4
N_CORES = 8
HPC = 8            # heads per core
VW = HEAD_DIM + 1  # v width per head incl ones column
MM_DT = mybir.dt.bfloat16
F32R = mybir.dt.float32r
F32 = mybir.dt.float32
BF16_NP = ml_dtypes.bfloat16

_CACHED = {}


def split_multi_waits(nc):
    """walrus in this container encodes at most ONE sync wait per
    instruction. First merge redundant waits (several sem-ge-imm waits on
    the SAME counting semaphore collapse to the max threshold); hoist any
    remaining extras onto same-engine NOPs placed just before."""
    fn = nc.m.functions[0]
    for bb in fn.blocks:
        new_list = []
        changed = False
        for inst in list(bb.instructions):
            si = getattr(inst, "sync_info", None)
            waits = list(si.on_wait) if (si and si.on_wait) else []
            if len(waits) > 1:
                merged = []
                best_ge = {}
                for w in waits:
                    if w.wait_mode == "sem-ge-imm" and w.wait_reg is None:
                        key = (w.sync_type, w.id)
                        cur = best_ge.get(key)
                        if cur is None or w.wait_value > cur.wait_value:
                            best_ge[key] = w
                    else:
                        merged.append(w)
                merged.extend(best_ge.values())
                waits = merged
                si.on_wait = list(waits)
            if len(waits) > 1:
                changed = True
                for w in waits[:-1]:
                    nop = mybir.InstNoOp(
                        name=f"waitnop-{nc.next_id()}", ins=[], outs=[]
                    )
                    nop.engine = inst.engine
                    nop.sync_info = mybir.SyncInfo(on_wait=[w], on_update=[])
                    nc.register_instruction(nop)
                    new_list.append(nop)
                si.on_wait = [waits[-1]]
            new_list.append(inst)
        if changed:
            bb.instructions = new_list


def build_nc(loop_reps=None, only=None):
    nc = bass.Bass()
    xT = nc.declare_dram_parameter("xT", [D_MODEL, T], MM_DT, isOutput=False)
    wqkT = nc.declare_dram_parameter("wqkT", [D_MODEL, 1024], MM_DT, isOutput=False)
    wqksT = nc.declare_dram_parameter("wqksT", [D_MODEL, 1024], MM_DT, isOutput=False)
    wvT = nc.declare_dram_parameter("wvT", [D_MODEL, HPC * VW], MM_DT, isOutput=False)
    woT = nc.declare_dram_parameter("woT", [HPC * HEAD_DIM, D_MODEL], MM_DT, isOutput=False)
    cosT = nc.declare_dram_parameter("cosT", [128, T], F32, isOutput=False)
    sinT = nc.declare_dram_parameter("sinT", [128, T], F32, isOutput=False)
    masks = nc.declare_dram_parameter("masks", [128, 2, 4, 512], MM_DT, isOutput=False)
    ones_d = nc.declare_dram_parameter("ones_d", [128, 128], F32, isOutput=False)
    ones_b = nc.declare_dram_parameter("ones_b", [128, 128], MM_DT, isOutput=False)
    yT = nc.declare_dram_parameter("yT", [D_MODEL, T], F32, isOutput=True)

    r = lambda ap: ap.bitcast(F32R)
    Exp = mybir.ActivationFunctionType.Exp
    xT_k = xT.rearrange("(kb p) t -> p kb t", p=128)      # [128, 8, T]
    wqkT_k = wqkT.rearrange("(kb p) f -> p kb f", p=128)  # [128, 8, 1024]
    wqksT_k = wqksT.rearrange("(kb p) f -> p kb f", p=128)
    wvT_k = wvT.rearrange("(kb p) f -> p kb f", p=128)    # [128, 8, 520]
    woT_k = woT.rearrange("(fb p) o -> p fb o", p=128)    # [128, 4, 1024]
    yT_k = yT.rearrange("(ob p) t -> p ob t", p=128)      # [128, 8, T]

    import contextlib
    with tile.TileContext(nc) as tc:
        with (tc.For_i(0, loop_reps) if loop_reps else contextlib.nullcontext()):
            with tc.tile_pool(name="persist", bufs=1) as pp:
                # a=0,1: q head-pairs (hp=2a+b); a=2,3: k head-pairs
                qk2 = [pp.tile([128, 2, T], MM_DT, tag=f"qk{a}", name=f"qk{a}")
                       for a in range(4)]
                v_all = pp.tile([128, 16, HPC * VW], MM_DT, tag="v_all", name="v_all")
                ones64 = pp.tile([1, 64], F32R, tag="ones64", name="ones64")
                nc.sync.dma_start(out=ones64, in_=r(ones_d[0:1, 0:64]))

                if only == "attn":
                    for a in range(4):
                        nc.sync.dma_start(out=qk2[a][:, 0, :], in_=xT_k[:, a, :])
                        nc.sync.dma_start(out=qk2[a][:, 1, :], in_=xT_k[:, 4 + a, :])
                    nc.sync.dma_start(out=v_all[:, 0:8, :], in_=wvT_k)
                    nc.sync.dma_start(out=v_all[:, 8:16, :], in_=wvT_k)
                    ones_ap0 = v_all.rearrange(
                        "p vg (h d) -> p vg h d", d=VW)[:, :, :, HEAD_DIM]
                    nc.sync.dma_start(
                        out=ones_ap0,
                        in_=ones_b.rearrange("p (vg h) -> p vg h", h=HPC))
                # ---------------- v projection pass ----------------
                _proj_phases = 0 if only == "attn" else 4
                with tc.tile_pool(name="wvp", bufs=1) as wvp, \
                     tc.tile_pool(name="xvp", bufs=2) as xvp, \
                     tc.tile_pool(name="ppsv", bufs=3, space="PSUM") as ppsv:
                    wv_all = wvp.tile([128, 8, HPC * VW], MM_DT, tag="wv", name="wv_all")
                    if _proj_phases:
                        nc.sync.dma_start(out=wv_all, in_=wvT_k)
                    for p in range(_proj_phases):
                        tsl = slice(512 * p, 512 * p + 512)
                        xt = xvp.tile([128, 8, 512], MM_DT, tag="xv", name="xv")
                        for kb in range(8):
                            nc.sync.dma_start(out=xt[:, kb, :], in_=xT_k[:, kb, tsl])
                        for vt in range(4):
                            vg = 4 * p + vt
                            vsl = slice(128 * vt, 128 * vt + 128)
                            psv = ppsv.tile([128, 2, 260], F32, tag="psv",
                                            padded_shape=[128, 2, 512])
                            for c in range(2):
                                for kb in range(8):
                                    nc.tensor.matmul(
                                        psv[:, c, :],
                                        xt[:, kb, vsl],
                                        wv_all[:, kb, 260 * c:260 * c + 260],
                                        start=(kb == 0),
                                        stop=(kb == 7),
                                    )
                            nc.vector.tensor_copy(v_all[:, vg, :], psv)
                    # ones columns for all 16 v tiles in one DMA
                    if _proj_phases:
                        ones_ap = v_all.rearrange(
                            "p vg (h d) -> p vg h d", d=VW
                        )[:, :, :, HEAD_DIM]
                        nc.sync.dma_start(
                            out=ones_ap,
                            in_=ones_b.rearrange("p (vg h) -> p vg h", h=HPC),
                        )

                # ---------------- q/k projection + rope pass ----------------
                # ps4 slots: 0,1 = m-pair straight proj; 2,3 = swapped proj.
                # rope = ps4[0:2]*cos + ps4[2:4]*sin, three DVE ops per pair.
                with tc.tile_pool(name="wqp", bufs=1) as wqp, \
                     tc.tile_pool(name="xqp", bufs=2) as xqp, \
                     tc.tile_pool(name="cspool", bufs=2) as csp, \
                     tc.tile_pool(name="ptmp", bufs=2) as ptmp, \
                     tc.tile_pool(name="pps", bufs=2, space="PSUM") as pps:
                    wqk_all = wqp.tile([128, 8, 1024], MM_DT, tag="wqk", name="wqk_all")
                    wqks_all = wqp.tile([128, 8, 1024], MM_DT, tag="wqks", name="wqks_all")
                    for kb in range(8 if _proj_phases else 0):
                        nc.sync.dma_start(out=wqk_all[:, kb, :], in_=wqkT_k[:, kb, :])
                        nc.sync.dma_start(out=wqks_all[:, kb, :], in_=wqksT_k[:, kb, :])
                    for p in range(_proj_phases):
                        tsl = slice(512 * p, 512 * p + 512)
                        xt = xqp.tile([128, 8, 512], MM_DT, tag="xq", name="xq")
                        for kb in range(8):
                            nc.sync.dma_start(out=xt[:, kb, :], in_=xT_k[:, kb, tsl])
                        cos2 = csp.tile([128, 2, 512], F32, tag="cos")
                        sin2 = csp.tile([128, 2, 512], F32, tag="sin")
                        for b in range(2):
                            nc.sync.dma_start(out=cos2[:, b, :], in_=cosT[:, tsl])
                            nc.sync.dma_start(out=sin2[:, b, :], in_=sinT[:, tsl])
                        for pg in range(4):
                            ps4 = pps.tile([128, 4, 512], F32, tag="ps4")
                            for sw in range(2):
                                w = wqk_all if sw == 0 else wqks_all
                                for mi in range(2):
                                    f0 = 128 * (2 * pg + mi)
                                    for kb in range(8):
                                        nc.tensor.matmul(
                                            ps4[:, 2 * sw + mi, :],
                                            w[:, kb, f0:f0 + 128],
                                            xt[:, kb, :],
                                            start=(kb == 0),
                                            stop=(kb == 7),
                                        )
                            rc2 = ptmp.tile([128, 2, 512], F32, tag="rc")
                            nc.vector.tensor_mul(rc2, ps4[:, 0:2, :], cos2)
                            tb2 = ptmp.tile([128, 2, 512], F32, tag="tb")
                            nc.vector.tensor_mul(tb2, ps4[:, 2:4, :], sin2)
                            nc.gpsimd.tensor_add(qk2[pg][:, :, tsl], rc2, tb2)

                if only == "proj":
                    for a in range(4):
                        nc.sync.dma_start(
                            out=yT_k[:, a, :].bitcast(MM_DT)[:, 0:T],
                            in_=qk2[a][:, 0, :])
                        nc.sync.dma_start(
                            out=yT_k[:, 4 + a, :].bitcast(MM_DT)[:, 0:T],
                            in_=qk2[a][:, 1, :])
                # ---------------- attention + out-proj ----------------
                # PSUM: sps tag "s" [128,2,512] bufs=3 (6 banks) + ops tag
                # "o" [65,2,512] bufs=1 (2 banks). bc2/y2 rotate through "s".
                with tc.tile_pool(name="wopool", bufs=1) as wop, \
                     tc.tile_pool(name="apool", bufs=2) as apool, \
                     tc.tile_pool(name="epool", bufs=4) as ep, \
                     tc.tile_pool(name="dtmp", bufs=3) as dtmp, \
                     tc.tile_pool(name="ypool", bufs=2) as yp, \
                     tc.tile_pool(name="sps", bufs=3, space="PSUM") as sps, \
                     tc.tile_pool(name="ops", bufs=1, space="PSUM") as ops:
                    wo_all = wop.tile([128, 4, 1024], MM_DT, tag="wo", name="wo_all")
                    masks_sb = wop.tile([128, 2, 4, 512], MM_DT, tag="masks", name="masks_sb")
                    if only != "proj":
                        nc.sync.dma_start(out=wo_all, in_=woT_k)
                        nc.sync.dma_start(out=masks_sb, in_=masks[:, :, :, :])

                    # One flat software pipeline over every (qt, hp, kb) unit:
                    # S(idx+2) is emitted before O(idx) so the PE never
                    # idle-waits on the ACT exp; divide chains and each qt's
                    # out-projection are deferred into later units (deferred
                    # actions) so the pipeline never drains at hp/qt
                    # boundaries.
                    qts = () if only == "proj" else (3, 2, 1, 0)
                    units = [(qt, hp, kb) for qt in qts
                             for hp in range(4) for kb in range(4 * (qt + 1))]
                    e2cache = {}
                    o2cache = {}
                    osb_cache = {}
                    aT_tiles = {}
                    actions = {}

                    def cols(qt, kb):
                        j = kb - 4 * qt
                        col0 = (0, 128, 256, 384)[j] if j >= 0 else 0
                        return j, slice(col0, 512)

                    def emit_S(qt, hp, kb):
                        qa, qb = hp // 2, hp % 2
                        qs0 = 512 * qt
                        j, csl = cols(qt, kb)
                        s2 = sps.tile([128, 2, 512], F32, tag="s")
                        for i in range(2):
                            rsl = slice(64 * i, 64 * i + 64)
                            nc.tensor.matmul(
                                s2[:, i, csl],
                                qk2[2 + qa][rsl, qb, 128 * kb:128 * kb + 128],
                                qk2[qa][rsl, qb, qs0 + csl.start:qs0 + 512],
                                start=True,
                                stop=True,
                            )
                        e2 = ep.tile([128, 2, 512], MM_DT, tag="e")
                        nc.scalar.activation(
                            e2[:, :, csl], s2[:, :, csl], Exp, scale=0.125
                        )
                        if j >= 0:
                            # masked product goes to a separate tile so the
                            # exp never has to wait on the DVE (keeps every
                            # exp wait on the PE semaphore only)
                            e2m = ep.tile([128, 2, 512], MM_DT, tag="em")
                            nc.gpsimd.tensor_mul(
                                e2m[:, :, csl], e2[:, :, csl],
                                masks_sb[:, :, j, csl]
                            )
                            e2 = e2m
                        e2cache[(qt, hp, kb)] = e2

                    def emit_O(qt, hp, kb):
                        _, csl = cols(qt, kb)
                        if kb == 0:
                            o2cache[(qt, hp)] = ops.tile([65, 2, 512], F32,
                                                         tag="o", name="o2")
                        e2 = e2cache.pop((qt, hp, kb))
                        for i in range(2):
                            h = 2 * hp + i
                            nc.tensor.matmul(
                                o2cache[(qt, hp)][:, i, csl],
                                v_all[:, kb, VW * h:VW * h + VW],
                                e2[:, i, csl],
                                start=(kb == 0),
                                stop=(kb == 4 * (qt + 1) - 1),
                            )

                    def emit_div_a(qt, hp):
                        # single DVE read frees the o2 buffer; row 64 =
                        # softmax denominators (bf16 is enough for them)
                        o_sb = dtmp.tile([65, 2, 512], MM_DT, tag="osb")
                        nc.vector.tensor_copy(o_sb, o2cache.pop((qt, hp)))
                        rsum2 = dtmp.tile([1, 2, 512], F32R, tag="rsum")
                        with nc.allow_low_precision("f32r is full-width"):
                            nc.vector.reciprocal(rsum2, o_sb[64:65, :, :])
                        osb_cache[(qt, hp)] = (o_sb, rsum2)

                    def emit_div_b(qt, hp):
                        o_sb, rsum2 = osb_cache.pop((qt, hp))
                        if qt not in aT_tiles:
                            aT_tiles[qt] = apool.tile([128, 4, 512], MM_DT,
                                                      tag="aT", name="aT")
                        bc2 = sps.tile([128, 2, 512], F32, tag="s", name="bc2")
                        for i in range(2):
                            nc.tensor.matmul(bc2[0:64, i, :], ones64,
                                             rsum2[:, i, :], start=True, stop=True)
                        for i in range(2):
                            nc.vector.tensor_mul(
                                aT_tiles[qt][64 * i:64 * i + 64, hp, :],
                                o_sb[0:64, i, :],
                                bc2[0:64, i, :],
                            )

                    def emit_y(qt):
                        aT = aT_tiles.pop(qt)
                        qs0 = 512 * qt
                        for yq in range(4):
                            y2 = sps.tile([128, 2, 512], F32, tag="s", name="y2")
                            for obi in range(2):
                                ob = 2 * yq + obi
                                for fb in range(4):
                                    nc.tensor.matmul(
                                        y2[:, obi, :],
                                        wo_all[:, fb, 128 * ob:128 * ob + 128],
                                        aT[:, fb, :],
                                        start=(fb == 0),
                                        stop=(fb == 3),
                                    )
                            y_all = yp.tile([128, 2, 512], F32, tag="y_all",
                                            name="y_all")
                            nc.vector.tensor_copy(y_all, y2)
                            nc.sync.dma_start(
                                out=yT_k[:, 2 * yq:2 * yq + 2, qs0:qs0 + 512],
                                in_=y_all,
                            )

                    if units:
                        emit_S(*units[0])
                        if len(units) > 1:
                            emit_S(*units[1])
                    for idx, (qt, hp, kb) in enumerate(units):
                        if idx + 2 < len(units):
                            emit_S(*units[idx + 2])
                        for fn in actions.pop(idx, ()):
                            fn()
                        emit_O(qt, hp, kb)
                        if kb == 4 * (qt + 1) - 1:  # last block of this hp
                            actions.setdefault(idx + 1, []).append(
                                lambda qt=qt, hp=hp: emit_div_a(qt, hp))
                            actions.setdefault(idx + 2, []).append(
                                lambda qt=qt, hp=hp: emit_div_b(qt, hp))
                            if hp == 3:
                                actions.setdefault(idx + 3, []).append(
                                    lambda qt=qt: emit_y(qt))
                    for idx in sorted(actions):
                        for fn in actions[idx]:
                            fn()

    split_multi_waits(nc)
    nc.finalize()
    return nc


def host_inputs(x, w_qkv, w_out):
    """Per-core input dicts."""
    x = np.asarray(x, dtype=np.float32)
    w_qkv = np.asarray(w_qkv, dtype=np.float32)
    w_out = np.asarray(w_out, dtype=np.float32)

    theta = 1.0 / (10000.0 ** (np.arange(0, HEAD_DIM, 2, dtype=np.float32) / HEAD_DIM))
    t = np.arange(T, dtype=np.float32)
    freqs = np.outer(t, theta)  # [T, 32]
    cos32 = np.cos(freqs).astype(np.float32).T  # [32, T]
    sin32 = np.sin(freqs).astype(np.float32).T
    cosT = np.tile(cos32, (4, 1))  # [128, T] rows r -> freq r%32
    sinT = np.tile(sin32, (4, 1))  # plain signed sin, same row layout as cos

    # rotate-half permutation with signs folded into the swapped weights:
    # new_col[f] = sign(f) * old_col[partner(f)], per 64-dim head block
    f = np.arange(1024)
    d = f % 64
    partner = np.where(d < 32, f + 32, f - 32)
    sign = np.where(d < 32, -1.0, 1.0).astype(np.float32)

    # canonical diagonal-block triangle (allow q_local >= k_local),
    # duplicated along a 2-wide head lane for paired-head tiles
    kl = np.arange(128)[:, None]
    ql = np.arange(128)[None, :]
    tri = (ql >= kl).astype(np.float32)  # [128, 128]
    masksf = np.ones((128, 2, 4, 512), dtype=np.float32)
    for j in range(4):
        masksf[:, :, j, 128 * j:128 * j + 128] = tri[:, None, :]
    masks = np.ascontiguousarray(masksf).astype(BF16_NP)

    maps = []
    for c in range(N_CORES):
        b, g = divmod(c, 2)
        heads = range(HPC * g, HPC * g + HPC)
        q_rows = np.concatenate([np.arange(64 * h, 64 * h + 64) for h in heads])
        wqkT = np.concatenate(
            [w_qkv[q_rows, :], w_qkv[1024 + q_rows, :]], axis=0
        ).T.copy()  # [1024 c, 1024 f]
        wqksT = wqkT[:, partner] * sign[None, :]
        wv = w_qkv[2048 + q_rows, :]  # [512, 1024]
        wvT = np.zeros((D_MODEL, HPC * VW), dtype=np.float32)
        for lh in range(HPC):
            wvT[:, VW * lh:VW * lh + HEAD_DIM] = wv[64 * lh:64 * lh + 64, :].T
        woT = w_out[:, q_rows].T.copy()  # [512, 1024]
        maps.append({
            "xT": np.ascontiguousarray(x[b].T).astype(BF16_NP),
            "wqkT": np.ascontiguousarray(wqkT).astype(BF16_NP),
            "wqksT": np.ascontiguousarray(wqksT).astype(BF16_NP),
            "wvT": wvT.astype(BF16_NP),
            "woT": np.ascontiguousarray(woT).astype(BF16_NP),
            "cosT": np.ascontiguousarray(cosT),
            "sinT": np.ascontiguousarray(sinT),
            "masks": masks,
            "ones_d": np.ones((128, 128), dtype=np.float32),
            "ones_b": np.ones((128, 128), dtype=BF16_NP),
        })
    return maps


def assemble(results):
    y = np.empty((B, T, D_MODEL), dtype=np.float32)
    for b in range(B):
        yT = results[2 * b]["yT"] + results[2 * b + 1]["yT"]
        y[b] = yT.T
    return y


def kernel(x, w_qkv, w_out):
    if "nc" not in _CACHED:
        _CACHED["nc"] = build_nc()
    nc = _CACHED["nc"]
    maps = host_inputs(x, w_qkv, w_out)
    res = run_bass_kernel_spmd(nc, maps, list(range(N_CORES)))
    return assemble(res.results)


# revision 20
# speedup vs baseline: 1.0210x; 1.0210x over previous
"""Causal self-attention (B=4, T=2048, D=1024, H=16, rope) on 8 trn2 cores.

Sharding: DP over batch (4) x TP over heads (2 groups of 8). Core c handles
batch c//2, heads [8*(c%2), 8*(c%2)+8). Host pre-slices/transposes weights,
sums the two partial out-projections per batch afterwards.

Per-core kernel (v8): bf16 matmul datapath (PSUM accumulation f32).
  - RoPE via a second projection against sign-folded swapped weights
    (wqksT[:,f] = sign(f) * wqkT[:,partner(f)]): q_rot = q*cos + q_swap*sin
    becomes three DVE ops per m-pair reading the projection PSUM directly --
    no ACT staging copy, no gpsimd partition-offset muls.
  - v projected token-major in its own pass (x re-streamed) [tok, 65*8] with
    a ones column per head so softmax denominators ride the attention matmul.
  - scores transposed (keys on partitions): S_T = k_blk.T @ q per 128-key
    block; 2-head-wide tiles, PSUM triple-buffered, S/exp/O software-
    pipelined (S of block idx+2 issues before O of block idx so the PE never
    idle-waits on the ACT exp); one ACT Exp per block (scale=1/8 folded) ->
    bf16. Causal: fully-masked blocks skipped, diagonal blocks column-trimmed
    to (0,128,256,384); the diagonal triangle is masked post-exp by a DVE mul
    into a SEPARATE tile so exps only ever wait on the PE semaphore.
  - O_T[d, q] accumulates over key blocks in PSUM; row 64 = softmax sums;
    divide via ones-matmul partition broadcast (f32r) + DVE reciprocal/mul.
  - out-projection per 512-token block -> yT [1024, 2048] partial (f32).
"""
import sys

for _p in ("/opt/trn_rl_repo", "/root/.axon_site/_ro/trn_rl_repo"):
    if _p not in sys.path:
        sys.path.insert(0, _p)

import numpy as np
import ml_dtypes
import concourse.bass as bass
import concourse.mybir as mybir
import concourse.tile as tile
from concourse.bass_utils import run_bass_kernel_spmd

D_MODEL = 1024
N_HEADS = 16
HEAD_DIM = 64
T = 2048
B = 4
N_CORES = 8
HPC = 8            # heads per core
VW = HEAD_DIM + 1  # v width per head incl ones column
MM_DT = mybir.dt.bfloat16
F32R = mybir.dt.float32r
F32 = mybir.dt.float32
BF16_NP = ml_dtypes.bfloat16

_CACHED = {}


def split_multi_waits(nc):
    """walrus in this container encodes at most ONE sync wait per
    instruction. First merge redundant waits (several sem-ge-imm waits on
    the SAME counting semaphore collapse to the max threshold); hoist any
    remaining extras onto same-engine NOPs placed just before."""
    fn = nc.m.functions[0]
    for bb in fn.blocks:
        new_list = []
        changed = False
        for inst in list(bb.instructions):
            si = getattr(inst, "sync_info", None)
            waits = list(si.on_wait) if (si and si.on_wait) else []
            if len(waits) > 1:
                merged = []
                best_ge = {}
                for w in waits:
                    if w.wait_mode == "sem-ge-imm" and w.wait_reg is None:
                        key = (w.sync_type, w.id)
                        cur = best_ge.get(key)
                        if cur is None or w.wait_value > cur.wait_value:
                            best_ge[key] = w
                    else:
                        merged.append(w)
                merged.extend(best_ge.values())
                waits = merged
                si.on_wait = list(waits)
            if len(waits) > 1:
                changed = True
                for w in waits[:-1]:
                    nop = mybir.InstNoOp(
                        name=f"waitnop-{nc.next_id()}", ins=[], outs=[]
                    )
                    nop.engine = inst.engine
                    nop.sync_info = mybir.SyncInfo(on_wait=[w], on_update=[])
                    nc.register_instruction(nop)
                    new_list.append(nop)
                si.on_wait = [waits[-1]]
            new_list.append(inst)
        if changed:
            bb.instructions = new_list


def build_nc(loop_reps=None, only=None):
    nc = bass.Bass()
    xT = nc.declare_dram_parameter("xT", [D_MODEL, T], MM_DT, isOutput=False)
    wqkT = nc.declare_dram_parameter("wqkT", [D_MODEL, 1024], MM_DT, isOutput=False)
    wqksT = nc.declare_dram_parameter("wqksT", [D_MODEL, 1024], MM_DT, isOutput=False)
    wvT = nc.declare_dram_parameter("wvT", [D_MODEL, HPC * VW], MM_DT, isOutput=False)
    woT = nc.declare_dram_parameter("woT", [HPC * HEAD_DIM, D_MODEL], MM_DT, isOutput=False)
    cosT = nc.declare_dram_parameter("cosT", [128, T], F32, isOutput=False)
    sinT = nc.declare_dram_parameter("sinT", [128, T], F32, isOutput=False)
    masks = nc.declare_dram_parameter("masks", [128, 2, 4, 512], MM_DT, isOutput=False)
    ones_d = nc.declare_dram_parameter("ones_d", [128, 128], F32, isOutput=False)
    ones_b = nc.declare_dram_parameter("ones_b", [128, 128], MM_DT, isOutput=False)
    yT = nc.declare_dram_parameter("yT", [D_MODEL, T], F32, isOutput=True)

    r = lambda ap: ap.bitcast(F32R)
    Exp = mybir.ActivationFunctionType.Exp
    xT_k = xT.rearrange("(kb p) t -> p kb t", p=128)      # [128, 8, T]
    wqkT_k = wqkT.rearrange("(kb p) f -> p kb f", p=128)  # [128, 8, 1024]
    wqksT_k = wqksT.rearrange("(kb p) f -> p kb f", p=128)
    wvT_k = wvT.rearrange("(kb p) f -> p kb f", p=128)    # [128, 8, 520]
    woT_k = woT.rearrange("(fb p) o -> p fb o", p=128)    # [128, 4, 1024]
    yT_k = yT.rearrange("(ob p) t -> p ob t", p=128)      # [128, 8, T]

    import contextlib
    with tile.TileContext(nc) as tc:
        with (tc.For_i(0, loop_reps) if loop_reps else contextlib.nullcontext()):
            with tc.tile_pool(name="persist", bufs=1) as pp:
                # a=0,1: q head-pairs (hp=2a+b); a=2,3: k head-pairs
                qk2 = [pp.tile([128, 2, T], MM_DT, tag=f"qk{a}", name=f"qk{a}")
                       for a in range(4)]
                v_all = pp.tile([128, 16, HPC * VW], MM_DT, tag="v_all", name="v_all")
                ones64 = pp.tile([1, 64], F32R, tag="ones64", name="ones64")
                nc.sync.dma_start(out=ones64, in_=r(ones_d[0:1, 0:64]))

                if only == "attn":
                    for a in range(4):
                        nc.sync.dma_start(out=qk2[a][:, 0, :], in_=xT_k[:, a, :])
                        nc.sync.dma_start(out=qk2[a][:, 1, :], in_=xT_k[:, 4 + a, :])
                    nc.sync.dma_start(out=v_all[:, 0:8, :], in_=wvT_k)
                    nc.sync.dma_start(out=v_all[:, 8:16, :], in_=wvT_k)
                    ones_ap0 = v_all.rearrange(
                        "p vg (h d) -> p vg h d", d=VW)[:, :, :, HEAD_DIM]
                    nc.sync.dma_start(
                        out=ones_ap0,
                        in_=ones_b.rearrange("p (vg h) -> p vg h", h=HPC))
                # ---------------- v projection pass ----------------
                _proj_phases = 0 if only == "attn" else 4
                with tc.tile_pool(name="wvp", bufs=1) as wvp, \
                     tc.tile_pool(name="xvp", bufs=2) as xvp, \
                     tc.tile_pool(name="ppsv", bufs=3, space="PSUM") as ppsv:
                    wv_all = wvp.tile([128, 8, HPC * VW], MM_DT, tag="wv", name="wv_all")
                    if _proj_phases:
                        nc.sync.dma_start(out=wv_all, in_=wvT_k)
                    for p in range(_proj_phases):
                        tsl = slice(512 * p, 512 * p + 512)
                        xt = xvp.tile([128, 8, 512], MM_DT, tag="xv", name="xv")
                        for kb in range(8):
                            nc.sync.dma_start(out=xt[:, kb, :], in_=xT_k[:, kb, tsl])
                        for vt in range(4):
                            vg = 4 * p + vt
                            vsl = slice(128 * vt, 128 * vt + 128)
                            psv = ppsv.tile([128, 2, 260], F32, tag="psv",
                                            padded_shape=[128, 2, 512])
                            for c in range(2):
                                for kb in range(8):
                                    nc.tensor.matmul(
                                        psv[:, c, :],
                                        xt[:, kb, vsl],
                                        wv_all[:, kb, 260 * c:260 * c + 260],
                                        start=(kb == 0),
                                        stop=(kb == 7),
                                    )
                            nc.vector.tensor_copy(v_all[:, vg, :], psv)
                    # ones columns for all 16 v tiles in one DMA
                    if _proj_phases:
                        ones_ap = v_all.rearrange(
                            "p vg (h d) -> p vg h d", d=VW
                        )[:, :, :, HEAD_DIM]
                        nc.sync.dma_start(
                            out=ones_ap,
                            in_=ones_b.rearrange("p (vg h) -> p vg h", h=HPC),
                        )

                # ---------------- q/k projection + rope pass ----------------
                # ps4 slots: 0,1 = m-pair straight proj; 2,3 = swapped proj.
                # rope = ps4[0:2]*cos + ps4[2:4]*sin, three DVE ops per pair.
                with tc.tile_pool(name="wqp", bufs=1) as wqp, \
                     tc.tile_pool(name="xqp", bufs=2) as xqp, \
                     tc.tile_pool(name="cspool", bufs=2) as csp, \
                     tc.tile_pool(name="ptmp", bufs=2) as ptmp, \
                     tc.tile_pool(name="pps", bufs=2, space="PSUM") as pps:
                    wqk_all = wqp.tile([128, 8, 1024], MM_DT, tag="wqk", name="wqk_all")
                    wqks_all = wqp.tile([128, 8, 1024], MM_DT, tag="wqks", name="wqks_all")
                    for kb in range(8 if _proj_phases else 0):
                        nc.sync.dma_start(out=wqk_all[:, kb, :], in_=wqkT_k[:, kb, :])
                        nc.sync.dma_start(out=wqks_all[:, kb, :], in_=wqksT_k[:, kb, :])
                    for p in range(_proj_phases):
                        tsl = slice(512 * p, 512 * p + 512)
                        xt = xqp.tile([128, 8, 512], MM_DT, tag="xq", name="xq")
                        for kb in range(8):
                            nc.sync.dma_start(out=xt[:, kb, :], in_=xT_k[:, kb, tsl])
                        cos2 = csp.tile([128, 2, 512], F32, tag="cos")
                        sin2 = csp.tile([128, 2, 512], F32, tag="sin")
                        for b in range(2):
                            nc.sync.dma_start(out=cos2[:, b, :], in_=cosT[:, tsl])
                            nc.sync.dma_start(out=sin2[:, b, :], in_=sinT[:, tsl])
                        for pg in range(4):
                            ps4 = pps.tile([128, 4, 512], F32, tag="ps4")
                            for sw in range(2):
                                w = wqk_all if sw == 0 else wqks_all
                                for mi in range(2):
                                    f0 = 128 * (2 * pg + mi)
                                    for kb in range(8):
                                        nc.tensor.matmul(
                                            ps4[:, 2 * sw + mi, :],
                                            w[:, kb, f0:f0 + 128],
                                            xt[:, kb, :],
                                            start=(kb == 0),
                                            stop=(kb == 7),
                                        )
                            rc2 = ptmp.tile([128, 2, 512], F32, tag="rc")
                            nc.vector.tensor_mul(rc2, ps4[:, 0:2, :], cos2)
                            tb2 = ptmp.tile([128, 2, 512], F32, tag="tb")
                            nc.vector.tensor_mul(tb2, ps4[:, 2:4, :], sin2)
                            nc.gpsimd.tensor_add(qk2[pg][:, :, tsl], rc2, tb2)

                if only == "proj":
                    for a in range(4):
                        nc.sync.dma_start(
                            out=yT_k[:, a, :].bitcast(MM_DT)[:, 0:T],
                            in_=qk2[a][:, 0, :])
                        nc.sync.dma_start(
                            out=yT_k[:, 4 + a, :].bitcast(MM_DT)[:, 0:T],
                            in_=qk2[a][:, 1, :])
                # ---------------- attention + out-proj ----------------
                # PSUM: sps tag "s" [128,2,512] bufs=3 (6 banks) + ops tag
                # "o" [65,2,512] bufs=1 (2 banks). bc2/y2 rotate through "s".
                with tc.tile_pool(name="wopool", bufs=1) as wop, \
                     tc.tile_pool(name="apool", bufs=2) as apool, \
                     tc.tile_pool(name="epool", bufs=6) as ep, \
                     tc.tile_pool(name="dtmp", bufs=4) as dtmp, \
                     tc.tile_pool(name="ypool", bufs=3) as yp, \
                     tc.tile_pool(name="sps", bufs=3, space="PSUM") as sps, \
                     tc.tile_pool(name="ops", bufs=1, space="PSUM") as ops:
                    wo_all = wop.tile([128, 4, 1024], MM_DT, tag="wo", name="wo_all")
                    masks_sb = wop.tile([128, 2, 4, 512], MM_DT, tag="masks", name="masks_sb")
                    if only != "proj":
                        nc.sync.dma_start(out=wo_all, in_=woT_k)
                        nc.sync.dma_start(out=masks_sb, in_=masks[:, :, :, :])

                    # One flat software pipeline over every (qt, hp, kb) unit:
                    # S(idx+2) is emitted before O(idx) so the PE never
                    # idle-waits on the ACT exp; divide chains and each qt's
                    # out-projection are deferred into later units (deferred
                    # actions) so the pipeline never drains at hp/qt
                    # boundaries.
                    qts = () if only == "proj" else (3, 2, 1, 0)
                    units = [(qt, hp, kb) for qt in qts
                             for hp in range(4) for kb in range(4 * (qt + 1))]
                    e2cache = {}
                    o2cache = {}
                    osb_cache = {}
                    aT_tiles = {}
                    actions = {}

                    def cols(qt, kb):
                        j = kb - 4 * qt
                        col0 = (0, 128, 256, 384)[j] if j >= 0 else 0
                        return j, slice(col0, 512)

                    def emit_S(qt, hp, kb):
                        qa, qb = hp // 2, hp % 2
                        qs0 = 512 * qt
                        j, csl = cols(qt, kb)
                        s2 = sps.tile([128, 2, 512], F32, tag="s")
                        for i in range(2):
                            rsl = slice(64 * i, 64 * i + 64)
                            nc.tensor.matmul(
                                s2[:, i, csl],
                                qk2[2 + qa][rsl, qb, 128 * kb:128 * kb + 128],
                                qk2[qa][rsl, qb, qs0 + csl.start:qs0 + 512],
                                start=True,
                                stop=True,
                            )
                        e2 = ep.tile([128, 2, 512], MM_DT, tag="e")
                        nc.scalar.activation(
                            e2[:, :, csl], s2[:, :, csl], Exp, scale=0.125
                        )
                        if j >= 0:
                            # masked product goes to a separate tile so the
                            # exp never has to wait on the DVE (keeps every
                            # exp wait on the PE semaphore only)
                            e2m = ep.tile([128, 2, 512], MM_DT, tag="em")
                            nc.vector.tensor_mul(
                                e2m[:, :, csl], e2[:, :, csl],
                                masks_sb[:, :, j, csl]
                            )
                            e2 = e2m
                        e2cache[(qt, hp, kb)] = e2

                    def emit_O(qt, hp, kb):
                        _, csl = cols(qt, kb)
                        if kb == 0:
                            o2cache[(qt, hp)] = ops.tile([65, 2, 512], F32,
                                                         tag="o", name="o2")
                        e2 = e2cache.pop((qt, hp, kb))
                        for i in range(2):
                            h = 2 * hp + i
                            nc.tensor.matmul(
                                o2cache[(qt, hp)][:, i, csl],
                                v_all[:, kb, VW * h:VW * h + VW],
                                e2[:, i, csl],
                                start=(kb == 0),
                                stop=(kb == 4 * (qt + 1) - 1),
                            )

                    def emit_div_a(qt, hp):
                        # single DVE read frees the o2 buffer; row 64 =
                        # softmax denominators (bf16 is enough for them)
                        o_sb = dtmp.tile([65, 2, 512], MM_DT, tag="osb")
                        nc.vector.tensor_copy(o_sb, o2cache.pop((qt, hp)))
                        rsum2 = dtmp.tile([1, 2, 512], F32R, tag="rsum")
                        with nc.allow_low_precision("f32r is full-width"):
                            nc.vector.reciprocal(rsum2, o_sb[64:65, :, :])
                        osb_cache[(qt, hp)] = (o_sb, rsum2)

                    def emit_div_b(qt, hp):
                        o_sb, rsum2 = osb_cache.pop((qt, hp))
                        if qt not in aT_tiles:
                            aT_tiles[qt] = apool.tile([128, 4, 512], MM_DT,
                                                      tag="aT", name="aT")
                        bc2 = sps.tile([128, 2, 512], F32, tag="s", name="bc2")
                        for i in range(2):
                            nc.tensor.matmul(bc2[0:64, i, :], ones64,
                                             rsum2[:, i, :], start=True, stop=True)
                        for i in range(2):
                            nc.vector.tensor_mul(
                                aT_tiles[qt][64 * i:64 * i + 64, hp, :],
                                o_sb[0:64, i, :],
                                bc2[0:64, i, :],
                            )

                    def emit_y(qt):
                        aT = aT_tiles.pop(qt)
                        qs0 = 512 * qt
                        for yq in range(4):
                            y2 = sps.tile([128, 2, 512], F32, tag="s", name="y2")
                            for obi in range(2):
                                ob = 2 * yq + obi
                                for fb in range(4):
                                    nc.tensor.matmul(
                                        y2[:, obi, :],
                                        wo_all[:, fb, 128 * ob:128 * ob + 128],
                                        aT[:, fb, :],
                                        start=(fb == 0),
                                        stop=(fb == 3),
                                    )
                            y_all = yp.tile([128, 2, 512], F32, tag="y_all",
                                            name="y_all")
                            nc.vector.tensor_copy(y_all, y2)
                            nc.sync.dma_start(
                                out=yT_k[:, 2 * yq:2 * yq + 2, qs0:qs0 + 512],
                                in_=y_all,
                            )

                    if units:
                        emit_S(*units[0])
                        if len(units) > 1:
                            emit_S(*units[1])
                    for idx, (qt, hp, kb) in enumerate(units):
                        if idx + 2 < len(units):
                            emit_S(*units[idx + 2])
                        for fn in actions.pop(idx, ()):
                            fn()
                        emit_O(qt, hp, kb)
                        if kb == 4 * (qt + 1) - 1:  # last block of this hp
                            actions.setdefault(idx + 1, []).append(
                                lambda qt=qt, hp=hp: emit_div_a(qt, hp))
                            actions.setdefault(idx + 2, []).append(
                                lambda qt=qt, hp=hp: emit_div_b(qt, hp))
                            if hp == 3:
                                actions.setdefault(idx + 3, []).append(
                                    lambda qt=qt: emit_y(qt))
                    for idx in sorted(actions):
                        for fn in actions[idx]:
                            fn()

    split_multi_waits(nc)
    nc.finalize()
    return nc


def host_inputs(x, w_qkv, w_out):
    """Per-core input dicts."""
    x = np.asarray(x, dtype=np.float32)
    w_qkv = np.asarray(w_qkv, dtype=np.float32)
    w_out = np.asarray(w_out, dtype=np.float32)

    theta = 1.0 / (10000.0 ** (np.arange(0, HEAD_DIM, 2, dtype=np.float32) / HEAD_DIM))
    t = np.arange(T, dtype=np.float32)
    freqs = np.outer(t, theta)  # [T, 32]
    cos32 = np.cos(freqs).astype(np.float32).T  # [32, T]
    sin32 = np.sin(freqs).astype(np.float32).T
    cosT = np.tile(cos32, (4, 1))  # [128, T] rows r -> freq r%32
    sinT = np.tile(sin32, (4, 1))  # plain signed sin, same row layout as cos

    # rotate-half permutation with signs folded into the swapped weights:
    # new_col[f] = sign(f) * old_col[partner(f)], per 64-dim head block
    f = np.arange(1024)
    d = f % 64
    partner = np.where(d < 32, f + 32, f - 32)
    sign = np.where(d < 32, -1.0, 1.0).astype(np.float32)

    # canonical diagonal-block triangle (allow q_local >= k_local),
    # duplicated along a 2-wide head lane for paired-head tiles
    kl = np.arange(128)[:, None]
    ql = np.arange(128)[None, :]
    tri = (ql >= kl).astype(np.float32)  # [128, 128]
    masksf = np.ones((128, 2, 4, 512), dtype=np.float32)
    for j in range(4):
        masksf[:, :, j, 128 * j:128 * j + 128] = tri[:, None, :]
    masks = np.ascontiguousarray(masksf).astype(BF16_NP)

    maps = []
    for c in range(N_CORES):
        b, g = divmod(c, 2)
        heads = range(HPC * g, HPC * g + HPC)
        q_rows = np.concatenate([np.arange(64 * h, 64 * h + 64) for h in heads])
        wqkT = np.concatenate(
            [w_qkv[q_rows, :], w_qkv[1024 + q_rows, :]], axis=0
        ).T.copy()  # [1024 c, 1024 f]
        wqksT = wqkT[:, partner] * sign[None, :]
        wv = w_qkv[2048 + q_rows, :]  # [512, 1024]
        wvT = np.zeros((D_MODEL, HPC * VW), dtype=np.float32)
        for lh in range(HPC):
            wvT[:, VW * lh:VW * lh + HEAD_DIM] = wv[64 * lh:64 * lh + 64, :].T
        woT = w_out[:, q_rows].T.copy()  # [512, 1024]
        maps.append({
            "xT": np.ascontiguousarray(x[b].T).astype(BF16_NP),
            "wqkT": np.ascontiguousarray(wqkT).astype(BF16_NP),
            "wqksT": np.ascontiguousarray(wqksT).astype(BF16_NP),
            "wvT": wvT.astype(BF16_NP),
            "woT": np.ascontiguousarray(woT).astype(BF16_NP),
            "cosT": np.ascontiguousarray(cosT),
            "sinT": np.ascontiguousarray(sinT),
            "masks": masks,
            "ones_d": np.ones((128, 128), dtype=np.float32),
            "ones_b": np.ones((128, 128), dtype=BF16_NP),
        })
    return maps


def assemble(results):
    y = np.empty((B, T, D_MODEL), dtype=np.float32)
    for b in range(B):
        yT = results[2 * b]["yT"] + results[2 * b + 1]["yT"]
        y[b] = yT.T
    return y


def kernel(x, w_qkv, w_out):
    if "nc" not in _CACHED:
        _CACHED["nc"] = build_nc()
    nc = _CACHED["nc"]
    maps = host_inputs(x, w_qkv, w_out)
    res = run_bass_kernel_spmd(nc, maps, list(range(N_CORES)))
    return assemble(res.results)


# revision 21
# speedup vs baseline: 1.0726x; 1.0506x over previous
"""Causal self-attention (B=4, T=2048, D=1024, H=16, rope) on 8 trn2 cores.

Sharding: DP over batch (4) x TP over heads (2 groups of 8). Core c handles
batch c//2, heads [8*(c%2), 8*(c%2)+8). Host pre-slices/transposes weights,
sums the two partial out-projections per batch afterwards.

Per-core kernel (v8): bf16 matmul datapath (PSUM accumulation f32).
  - RoPE via a second projection against sign-folded swapped weights
    (wqksT[:,f] = sign(f) * wqkT[:,partner(f)]): q_rot = q*cos + q_swap*sin
    becomes three DVE ops per m-pair reading the projection PSUM directly --
    no ACT staging copy, no gpsimd partition-offset muls.
  - v projected token-major in its own pass (x re-streamed) [tok, 65*8] with
    a ones column per head so softmax denominators ride the attention matmul.
  - scores transposed (keys on partitions): S_T = k_blk.T @ q per 128-key
    block; 2-head-wide tiles, PSUM triple-buffered, S/exp/O software-
    pipelined (S of block idx+2 issues before O of block idx so the PE never
    idle-waits on the ACT exp); one ACT Exp per block (scale=1/8 folded) ->
    bf16. Causal: fully-masked blocks skipped, diagonal blocks column-trimmed
    to (0,128,256,384); the diagonal triangle is masked post-exp by a DVE mul
    into a SEPARATE tile so exps only ever wait on the PE semaphore.
  - O_T[d, q] accumulates over key blocks in PSUM; row 64 = softmax sums;
    divide via ones-matmul partition broadcast (f32r) + DVE reciprocal/mul.
  - out-projection per 512-token block -> yT [1024, 2048] partial (f32).
"""
import sys

for _p in ("/opt/trn_rl_repo", "/root/.axon_site/_ro/trn_rl_repo"):
    if _p not in sys.path:
        sys.path.insert(0, _p)

import numpy as np
import ml_dtypes
import concourse.bass as bass
import concourse.mybir as mybir
import concourse.tile as tile
from concourse.bass_utils import run_bass_kernel_spmd

D_MODEL = 1024
N_HEADS = 16
HEAD_DIM = 64
T = 2048
B = 4
N_CORES = 8
HPC = 8            # heads per core
VW = HEAD_DIM + 1  # v width per head incl ones column
MM_DT = mybir.dt.bfloat16
F32R = mybir.dt.float32r
F32 = mybir.dt.float32
BF16_NP = ml_dtypes.bfloat16

_CACHED = {}


def split_multi_waits(nc):
    """walrus in this container encodes at most ONE sync wait per
    instruction. First merge redundant waits (several sem-ge-imm waits on
    the SAME counting semaphore collapse to the max threshold); hoist any
    remaining extras onto same-engine NOPs placed just before."""
    fn = nc.m.functions[0]
    for bb in fn.blocks:
        new_list = []
        changed = False
        for inst in list(bb.instructions):
            si = getattr(inst, "sync_info", None)
            waits = list(si.on_wait) if (si and si.on_wait) else []
            if len(waits) > 1:
                merged = []
                best_ge = {}
                for w in waits:
                    if w.wait_mode == "sem-ge-imm" and w.wait_reg is None:
                        key = (w.sync_type, w.id)
                        cur = best_ge.get(key)
                        if cur is None or w.wait_value > cur.wait_value:
                            best_ge[key] = w
                    else:
                        merged.append(w)
                merged.extend(best_ge.values())
                waits = merged
                si.on_wait = list(waits)
            if len(waits) > 1:
                changed = True
                for w in waits[:-1]:
                    nop = mybir.InstNoOp(
                        name=f"waitnop-{nc.next_id()}", ins=[], outs=[]
                    )
                    nop.engine = inst.engine
                    nop.sync_info = mybir.SyncInfo(on_wait=[w], on_update=[])
                    nc.register_instruction(nop)
                    new_list.append(nop)
                si.on_wait = [waits[-1]]
            new_list.append(inst)
        if changed:
            bb.instructions = new_list


def build_nc(loop_reps=None, only=None):
    nc = bass.Bass()
    xT = nc.declare_dram_parameter("xT", [D_MODEL, T], MM_DT, isOutput=False)
    wqkT = nc.declare_dram_parameter("wqkT", [D_MODEL, 1024], MM_DT, isOutput=False)
    wqksT = nc.declare_dram_parameter("wqksT", [D_MODEL, 1024], MM_DT, isOutput=False)
    wvT = nc.declare_dram_parameter("wvT", [D_MODEL, HPC * VW], MM_DT, isOutput=False)
    woT = nc.declare_dram_parameter("woT", [HPC * HEAD_DIM, D_MODEL], MM_DT, isOutput=False)
    cosT = nc.declare_dram_parameter("cosT", [128, T], F32, isOutput=False)
    sinT = nc.declare_dram_parameter("sinT", [128, T], F32, isOutput=False)
    masks = nc.declare_dram_parameter("masks", [128, 2, 4, 512], MM_DT, isOutput=False)
    ones_d = nc.declare_dram_parameter("ones_d", [128, 128], F32, isOutput=False)
    ones_b = nc.declare_dram_parameter("ones_b", [128, 128], MM_DT, isOutput=False)
    yT = nc.declare_dram_parameter("yT", [D_MODEL, T], F32, isOutput=True)

    r = lambda ap: ap.bitcast(F32R)
    Exp = mybir.ActivationFunctionType.Exp
    xT_k = xT.rearrange("(kb p) t -> p kb t", p=128)      # [128, 8, T]
    wqkT_k = wqkT.rearrange("(kb p) f -> p kb f", p=128)  # [128, 8, 1024]
    wqksT_k = wqksT.rearrange("(kb p) f -> p kb f", p=128)
    wvT_k = wvT.rearrange("(kb p) f -> p kb f", p=128)    # [128, 8, 520]
    woT_k = woT.rearrange("(fb p) o -> p fb o", p=128)    # [128, 4, 1024]
    yT_k = yT.rearrange("(ob p) t -> p ob t", p=128)      # [128, 8, T]

    import contextlib
    with tile.TileContext(nc) as tc:
        with (tc.For_i(0, loop_reps) if loop_reps else contextlib.nullcontext()):
            with tc.tile_pool(name="persist", bufs=1) as pp:
                # a=0,1: q head-pairs (hp=2a+b); a=2,3: k head-pairs
                qk2 = [pp.tile([128, 2, T], MM_DT, tag=f"qk{a}", name=f"qk{a}")
                       for a in range(4)]
                v_all = pp.tile([128, 16, HPC * VW], MM_DT, tag="v_all", name="v_all")
                ones64 = pp.tile([1, 64], F32R, tag="ones64", name="ones64")
                nc.sync.dma_start(out=ones64, in_=r(ones_d[0:1, 0:64]))

                if only == "attn":
                    for a in range(4):
                        nc.sync.dma_start(out=qk2[a][:, 0, :], in_=xT_k[:, a, :])
                        nc.sync.dma_start(out=qk2[a][:, 1, :], in_=xT_k[:, 4 + a, :])
                    nc.sync.dma_start(out=v_all[:, 0:8, :], in_=wvT_k)
                    nc.sync.dma_start(out=v_all[:, 8:16, :], in_=wvT_k)
                    ones_ap0 = v_all.rearrange(
                        "p vg (h d) -> p vg h d", d=VW)[:, :, :, HEAD_DIM]
                    nc.sync.dma_start(
                        out=ones_ap0,
                        in_=ones_b.rearrange("p (vg h) -> p vg h", h=HPC))
                # ---------------- v projection pass ----------------
                _proj_phases = 0 if only == "attn" else 4
                with tc.tile_pool(name="wvp", bufs=1) as wvp, \
                     tc.tile_pool(name="xvp", bufs=2) as xvp, \
                     tc.tile_pool(name="ppsv", bufs=3, space="PSUM") as ppsv:
                    wv_all = wvp.tile([128, 8, HPC * VW], MM_DT, tag="wv", name="wv_all")
                    if _proj_phases:
                        nc.sync.dma_start(out=wv_all, in_=wvT_k)
                    for p in range(_proj_phases):
                        tsl = slice(512 * p, 512 * p + 512)
                        xt = xvp.tile([128, 8, 512], MM_DT, tag="xv", name="xv")
                        for kb in range(8):
                            nc.sync.dma_start(out=xt[:, kb, :], in_=xT_k[:, kb, tsl])
                        for vt in range(4):
                            vg = 4 * p + vt
                            vsl = slice(128 * vt, 128 * vt + 128)
                            psv = ppsv.tile([128, 2, 260], F32, tag="psv",
                                            padded_shape=[128, 2, 512])
                            for c in range(2):
                                for kb in range(8):
                                    nc.tensor.matmul(
                                        psv[:, c, :],
                                        xt[:, kb, vsl],
                                        wv_all[:, kb, 260 * c:260 * c + 260],
                                        start=(kb == 0),
                                        stop=(kb == 7),
                                    )
                            nc.vector.tensor_copy(v_all[:, vg, :], psv)
                    # ones columns for all 16 v tiles in one DMA
                    if _proj_phases:
                        ones_ap = v_all.rearrange(
                            "p vg (h d) -> p vg h d", d=VW
                        )[:, :, :, HEAD_DIM]
                        nc.sync.dma_start(
                            out=ones_ap,
                            in_=ones_b.rearrange("p (vg h) -> p vg h", h=HPC),
                        )

                # ---------------- q/k projection + rope pass ----------------
                # ps4 slots: 0,1 = m-pair straight proj; 2,3 = swapped proj.
                # rope = ps4[0:2]*cos + ps4[2:4]*sin, three DVE ops per pair.
                with tc.tile_pool(name="wqp", bufs=1) as wqp, \
                     tc.tile_pool(name="xqp", bufs=2) as xqp, \
                     tc.tile_pool(name="cspool", bufs=2) as csp, \
                     tc.tile_pool(name="ptmp", bufs=2) as ptmp, \
                     tc.tile_pool(name="pps", bufs=2, space="PSUM") as pps:
                    wqk_all = wqp.tile([128, 8, 1024], MM_DT, tag="wqk", name="wqk_all")
                    wqks_all = wqp.tile([128, 8, 1024], MM_DT, tag="wqks", name="wqks_all")
                    for kb in range(8 if _proj_phases else 0):
                        nc.sync.dma_start(out=wqk_all[:, kb, :], in_=wqkT_k[:, kb, :])
                        nc.sync.dma_start(out=wqks_all[:, kb, :], in_=wqksT_k[:, kb, :])
                    for p in range(_proj_phases):
                        tsl = slice(512 * p, 512 * p + 512)
                        xt = xqp.tile([128, 8, 512], MM_DT, tag="xq", name="xq")
                        for kb in range(8):
                            nc.sync.dma_start(out=xt[:, kb, :], in_=xT_k[:, kb, tsl])
                        cos2 = csp.tile([128, 2, 512], F32, tag="cos")
                        sin2 = csp.tile([128, 2, 512], F32, tag="sin")
                        for b in range(2):
                            nc.sync.dma_start(out=cos2[:, b, :], in_=cosT[:, tsl])
                            nc.sync.dma_start(out=sin2[:, b, :], in_=sinT[:, tsl])
                        for pg in range(4):
                            ps4 = pps.tile([128, 4, 512], F32, tag="ps4")
                            for sw in range(2):
                                w = wqk_all if sw == 0 else wqks_all
                                for mi in range(2):
                                    f0 = 128 * (2 * pg + mi)
                                    for kb in range(8):
                                        nc.tensor.matmul(
                                            ps4[:, 2 * sw + mi, :],
                                            w[:, kb, f0:f0 + 128],
                                            xt[:, kb, :],
                                            start=(kb == 0),
                                            stop=(kb == 7),
                                        )
                            rc2 = ptmp.tile([128, 2, 512], F32, tag="rc")
                            nc.vector.tensor_mul(rc2, ps4[:, 0:2, :], cos2)
                            tb2 = ptmp.tile([128, 2, 512], F32, tag="tb")
                            nc.vector.tensor_mul(tb2, ps4[:, 2:4, :], sin2)
                            nc.gpsimd.tensor_add(qk2[pg][:, :, tsl], rc2, tb2)

                if only == "proj":
                    for a in range(4):
                        nc.sync.dma_start(
                            out=yT_k[:, a, :].bitcast(MM_DT)[:, 0:T],
                            in_=qk2[a][:, 0, :])
                        nc.sync.dma_start(
                            out=yT_k[:, 4 + a, :].bitcast(MM_DT)[:, 0:T],
                            in_=qk2[a][:, 1, :])
                # ---------------- attention + out-proj ----------------
                # PSUM: sps tag "s" [128,2,512] bufs=3 (6 banks) + ops tag
                # "o" [65,2,512] bufs=1 (2 banks). bc2/y2 rotate through "s".
                with tc.tile_pool(name="wopool", bufs=1) as wop, \
                     tc.tile_pool(name="apool", bufs=2) as apool, \
                     tc.tile_pool(name="epool", bufs=4) as ep, \
                     tc.tile_pool(name="dtmp", bufs=3) as dtmp, \
                     tc.tile_pool(name="ypool", bufs=2) as yp, \
                     tc.tile_pool(name="sps", bufs=3, space="PSUM") as sps, \
                     tc.tile_pool(name="ops", bufs=1, space="PSUM") as ops:
                    wo_all = wop.tile([128, 4, 1024], MM_DT, tag="wo", name="wo_all")
                    masks_sb = wop.tile([128, 2, 4, 512], MM_DT, tag="masks", name="masks_sb")
                    if only != "proj":
                        nc.sync.dma_start(out=wo_all, in_=woT_k)
                        nc.sync.dma_start(out=masks_sb, in_=masks[:, :, :, :])

                    # One flat software pipeline over every (qt, hp, kb) unit:
                    # S(idx+2) is emitted before O(idx) so the PE never
                    # idle-waits on the ACT exp; divide chains and each qt's
                    # out-projection are deferred into later units (deferred
                    # actions) so the pipeline never drains at hp/qt
                    # boundaries.
                    qts = () if only == "proj" else (3, 2, 1, 0)
                    units = [(qt, hp, kb) for qt in qts
                             for hp in range(4) for kb in range(4 * (qt + 1))]
                    e2cache = {}
                    o2cache = {}
                    osb_cache = {}
                    aT_tiles = {}
                    actions = {}

                    def cols(qt, kb):
                        j = kb - 4 * qt
                        col0 = (0, 128, 256, 384)[j] if j >= 0 else 0
                        return j, slice(col0, 512)

                    def emit_S(qt, hp, kb):
                        qa, qb = hp // 2, hp % 2
                        qs0 = 512 * qt
                        j, csl = cols(qt, kb)
                        s2 = sps.tile([128, 2, 512], F32, tag="s")
                        for i in range(2):
                            rsl = slice(64 * i, 64 * i + 64)
                            nc.tensor.matmul(
                                s2[:, i, csl],
                                qk2[2 + qa][rsl, qb, 128 * kb:128 * kb + 128],
                                qk2[qa][rsl, qb, qs0 + csl.start:qs0 + 512],
                                start=True,
                                stop=True,
                            )
                        e2 = ep.tile([128, 2, 512], MM_DT, tag="e")
                        nc.scalar.activation(
                            e2[:, :, csl], s2[:, :, csl], Exp, scale=0.125
                        )
                        if j >= 0:
                            # masked product goes to a separate tile so the
                            # exp never has to wait on the DVE (keeps every
                            # exp wait on the PE semaphore only)
                            e2m = ep.tile([128, 2, 512], MM_DT, tag="em")
                            nc.vector.tensor_mul(
                                e2m[:, :, csl], e2[:, :, csl],
                                masks_sb[:, :, j, csl]
                            )
                            e2 = e2m
                        e2cache[(qt, hp, kb)] = e2

                    def emit_O(qt, hp, kb):
                        _, csl = cols(qt, kb)
                        if kb == 0:
                            o2cache[(qt, hp)] = ops.tile([65, 2, 512], F32,
                                                         tag="o", name="o2")
                        e2 = e2cache.pop((qt, hp, kb))
                        for i in range(2):
                            h = 2 * hp + i
                            nc.tensor.matmul(
                                o2cache[(qt, hp)][:, i, csl],
                                v_all[:, kb, VW * h:VW * h + VW],
                                e2[:, i, csl],
                                start=(kb == 0),
                                stop=(kb == 4 * (qt + 1) - 1),
                            )

                    def emit_div_a(qt, hp):
                        # single DVE read frees the o2 buffer; row 64 =
                        # softmax denominators (bf16 is enough for them)
                        o_sb = dtmp.tile([65, 2, 512], MM_DT, tag="osb")
                        nc.vector.tensor_copy(o_sb, o2cache.pop((qt, hp)))
                        rsum2 = dtmp.tile([1, 2, 512], F32R, tag="rsum")
                        with nc.allow_low_precision("f32r is full-width"):
                            nc.vector.reciprocal(rsum2, o_sb[64:65, :, :])
                        osb_cache[(qt, hp)] = (o_sb, rsum2)

                    def emit_div_b(qt, hp):
                        o_sb, rsum2 = osb_cache.pop((qt, hp))
                        if qt not in aT_tiles:
                            aT_tiles[qt] = apool.tile([128, 4, 512], MM_DT,
                                                      tag="aT", name="aT")
                        bc2 = sps.tile([128, 2, 512], F32, tag="s", name="bc2")
                        for i in range(2):
                            nc.tensor.matmul(bc2[0:64, i, :], ones64,
                                             rsum2[:, i, :], start=True, stop=True)
                        for i in range(2):
                            nc.vector.tensor_mul(
                                aT_tiles[qt][64 * i:64 * i + 64, hp, :],
                                o_sb[0:64, i, :],
                                bc2[0:64, i, :],
                            )

                    def emit_y(qt):
                        aT = aT_tiles.pop(qt)
                        qs0 = 512 * qt
                        for yq in range(4):
                            y2 = sps.tile([128, 2, 512], F32, tag="s", name="y2")
                            for obi in range(2):
                                ob = 2 * yq + obi
                                for fb in range(4):
                                    nc.tensor.matmul(
                                        y2[:, obi, :],
                                        wo_all[:, fb, 128 * ob:128 * ob + 128],
                                        aT[:, fb, :],
                                        start=(fb == 0),
                                        stop=(fb == 3),
                                    )
                            y_all = yp.tile([128, 2, 512], F32, tag="y_all",
                                            name="y_all")
                            nc.vector.tensor_copy(y_all, y2)
                            nc.sync.dma_start(
                                out=yT_k[:, 2 * yq:2 * yq + 2, qs0:qs0 + 512],
                                in_=y_all,
                            )

                    if units:
                        emit_S(*units[0])
                        if len(units) > 1:
                            emit_S(*units[1])
                    for idx, (qt, hp, kb) in enumerate(units):
                        if idx + 2 < len(units):
                            emit_S(*units[idx + 2])
                        for fn in actions.pop(idx, ()):
                            fn()
                        emit_O(qt, hp, kb)
                        if kb == 4 * (qt + 1) - 1:  # last block of this hp
                            actions.setdefault(idx + 1, []).append(
                                lambda qt=qt, hp=hp: emit_div_a(qt, hp))
                            actions.setdefault(idx + 2, []).append(
                                lambda qt=qt, hp=hp: emit_div_b(qt, hp))
                            if hp == 3:
                                actions.setdefault(idx + 3, []).append(
                                    lambda qt=qt: emit_y(qt))
                    for idx in sorted(actions):
                        for fn in actions[idx]:
                            fn()

    split_multi_waits(nc)
    nc.finalize()
    return nc


def host_inputs(x, w_qkv, w_out):
    """Per-core input dicts."""
    x = np.asarray(x, dtype=np.float32)
    w_qkv = np.asarray(w_qkv, dtype=np.float32)
    w_out = np.asarray(w_out, dtype=np.float32)

    theta = 1.0 / (10000.0 ** (np.arange(0, HEAD_DIM, 2, dtype=np.float32) / HEAD_DIM))
    t = np.arange(T, dtype=np.float32)
    freqs = np.outer(t, theta)  # [T, 32]
    cos32 = np.cos(freqs).astype(np.float32).T  # [32, T]
    sin32 = np.sin(freqs).astype(np.float32).T
    cosT = np.tile(cos32, (4, 1))  # [128, T] rows r -> freq r%32
    sinT = np.tile(sin32, (4, 1))  # plain signed sin, same row layout as cos

    # rotate-half permutation with signs folded into the swapped weights:
    # new_col[f] = sign(f) * old_col[partner(f)], per 64-dim head block
    f = np.arange(1024)
    d = f % 64
    partner = np.where(d < 32, f + 32, f - 32)
    sign = np.where(d < 32, -1.0, 1.0).astype(np.float32)

    # canonical diagonal-block triangle (allow q_local >= k_local),
    # duplicated along a 2-wide head lane for paired-head tiles
    kl = np.arange(128)[:, None]
    ql = np.arange(128)[None, :]
    tri = (ql >= kl).astype(np.float32)  # [128, 128]
    masksf = np.ones((128, 2, 4, 512), dtype=np.float32)
    for j in range(4):
        masksf[:, :, j, 128 * j:128 * j + 128] = tri[:, None, :]
    masks = np.ascontiguousarray(masksf).astype(BF16_NP)

    maps = []
    for c in range(N_CORES):
        b, g = divmod(c, 2)
        heads = range(HPC * g, HPC * g + HPC)
        q_rows = np.concatenate([np.arange(64 * h, 64 * h + 64) for h in heads])
        wqkT = np.concatenate(
            [w_qkv[q_rows, :], w_qkv[1024 + q_rows, :]], axis=0
        ).T.copy()  # [1024 c, 1024 f]
        wqksT = wqkT[:, partner] * sign[None, :]
        wv = w_qkv[2048 + q_rows, :]  # [512, 1024]
        wvT = np.zeros((D_MODEL, HPC * VW), dtype=np.float32)
        for lh in range(HPC):
            wvT[:, VW * lh:VW * lh + HEAD_DIM] = wv[64 * lh:64 * lh + 64, :].T
        woT = w_out[:, q_rows].T.copy()  # [512, 1024]
        maps.append({
            "xT": np.ascontiguousarray(x[b].T).astype(BF16_NP),
            "wqkT": np.ascontiguousarray(wqkT).astype(BF16_NP),
            "wqksT": np.ascontiguousarray(wqksT).astype(BF16_NP),
            "wvT": wvT.astype(BF16_NP),
            "woT": np.ascontiguousarray(woT).astype(BF16_NP),
            "cosT": np.ascontiguousarray(cosT),
            "sinT": np.ascontiguousarray(sinT),
            "masks": masks,
            "ones_d": np.ones((128, 128), dtype=np.float32),
            "ones_b": np.ones((128, 128), dtype=BF16_NP),
        })
    return maps


def assemble(results):
    y = np.empty((B, T, D_MODEL), dtype=np.float32)
    for b in range(B):
        yT = results[2 * b]["yT"] + results[2 * b + 1]["yT"]
        y[b] = yT.T
    return y


def kernel(x, w_qkv, w_out):
    if "nc" not in _CACHED:
        _CACHED["nc"] = build_nc()
    nc = _CACHED["nc"]
    maps = host_inputs(x, w_qkv, w_out)
    res = run_bass_kernel_spmd(nc, maps, list(range(N_CORES)))
    return assemble(res.results)
